# revision 4
# baseline (speedup 1.0000x reference)
"""BatchBlur: depthwise 15x15 conv with per-sample kernels, reflection pad 7.

x: (32, 3, 512, 512) f32, kernel: (32, 15, 15) f32 -> out (32, 3, 512, 512) f32.

Strategy: pure data parallel over batch, 4 samples (12 channel-images) per
core on 8 cores. Host: reflection-pad x to (., 526, 526), store rows padded
to 528 with zero columns, cast to fp16, and build dual-band matrices
A[s, k, j, m]: for k < 46, A = kern[s, k-m, 2j]; for k >= 46,
A = kern[s, k-46-m, 2j+1] (band condition 0 <= dy < 15; j = 7 upper band
is zero).

Device: measured PE law on TRN2 - a matmul costs N * M_pad/128 cycles
(M_pad = output partitions padded to 32/64/128), independent of K, and
back-to-back matmuls accumulating into the SAME PSUM region stall ~112
extra cycles unless independent chains are interleaved. So: M=32 strips
(column-tiled PE, 4 tiles), dual-band K=92 (rows at column offsets 0 and
+1, so one stream covers taps 2j and 2j+1 -> 8 streams instead of 15),
and the 4 column tiles process 4 strips of one sample's image round-robin,
hiding the accumulation drain:
  out[m, n] += sum_k A[k, j, m] * rhs[k, n + 2j]
Cost: 12 img x 16 strips x 8 streams = 1536 matmuls x ~128 cycles ~ 83 us.
526 = 16*32 + 14 -> 16 uniform strips exactly cover an image. PSUM: one
f32 bank per step (4 strips x 32 rows = 128 partitions). Eviction casts to
fp16 (DVE) and stores fp16 in 128-row batches; the host casts back to f32
(adds ~4e-4 relative error).
"""
import os
import sys

for _p in ("/opt/trn_rl_repo", "/root/.axon_site/_ro/trn_rl_repo"):
    if _p not in sys.path and os.path.isdir(_p):
        sys.path.insert(0, _p)

import numpy as np

import concourse.bass as bass
import concourse.mybir as mybir
import concourse.tile as tile
from concourse import bacc
from concourse.bass_utils import run_bass_kernel_spmd

L = 15           # blur kernel size
P = L // 2       # reflection pad
B, C, H, W = 32, 3, 512, 512
N_CORES = 8
BS = B // N_CORES            # samples per core (4)
NIMG = BS * C                # channel images per core (12)
HP, WP = H + 2 * P, W + 2 * P  # 526
WPH = WP + 2                 # host row pitch: +2 zero cols so the +1-shifted
                             # band reads defined data at its last column
M = 32                       # output rows per strip (column-tile width)
KG = M + L - 1               # 46 rows per band
K2 = 2 * KG                  # 92 = dual-band contraction size
NSTRIP = H // M              # 16 strips per image, exact: HP = NSTRIP*M+14
QB = 4                       # strips per load/store batch
N_DX = (L + 1) // 2          # 8 streams (2 taps each; last is single)
N_WARMUP = 100               # dummy matmuls to release the HAM clock gate

F16 = mybir.dt.float16
F32 = mybir.dt.float32

_program_cache = None


def _build_program():
    nc = bacc.Bacc("TRN2", target_bir_lowering=False, debug=False)
    xp_d = nc.dram_tensor("xp", [NIMG, HP, WPH], F16,
                          kind="ExternalInput").ap()
    a_d = nc.dram_tensor("a", [BS, 128, N_DX, M], F16,
                         kind="ExternalInput").ap()
    out_d = nc.dram_tensor("out", [NIMG, H, W], F16,
                           kind="ExternalOutput").ap()

    # step order: image t of each sample, 4-strip batch b, strip q within it.
    # Column tile c always works on sample c (image 3c+t), so its weight
    # matrix never changes.
    steps = [(t, b) for t in range(C) for b in range(NSTRIP // QB)]

    with tile.TileContext(nc) as tc:
        with (
            tc.tile_pool(name="aconst", bufs=1) as apool,
            tc.tile_pool(name="warm", bufs=1) as wpool,
            tc.tile_pool(name="xin", bufs=3) as xpool,
            tc.tile_pool(name="oout", bufs=2) as opool,
            tc.tile_pool(name="psum", bufs=4, space="PSUM") as psum,
            tc.tile_pool(name="psumw", bufs=1, space="PSUM") as psumw,
        ):
            # HAM warm-up: a burst of matmuls on a zeroed scratch tile
            # releases the PE clock gate while the first input DMAs are in
            # flight.
            wsrc = wpool.tile([128, 64], mybir.dt.bfloat16)
            nc.gpsimd.memset(wsrc[:], 0.0)
            wacc = psumw.tile([64, 64], F32)
            for _ in range(N_WARMUP):
                nc.tensor.matmul(wacc[:], wsrc[:, :64], wsrc[:], start=True,
                                 stop=True)

            def load_batch(xt, c, t, b):
                # two DMAs per batch tile: lower band (rows at column offset
                # 0, Sync queue) and upper band (same rows at offset +1,
                # GpSimd queue), QB strips in free-dim blocks of WP columns
                img = 3 * c + t
                base = (img * HP + b * QB * M) * WPH
                nc.sync.dma_start(
                    out=xt[0:KG, :].rearrange("p (q c) -> p q c", c=WP),
                    in_=bass.AP(xp_d.tensor, base,
                                [[WPH, KG], [M * WPH, QB], [1, WP]]))
                nc.gpsimd.dma_start(
                    out=xt[KG:K2, :].rearrange("p (q c) -> p q c", c=WP),
                    in_=bass.AP(xp_d.tensor, base + 1,
                                [[WPH, KG], [M * WPH, QB], [1, WP]]))

            xt = {}
            for u in range(2):
                t, b = steps[u]
                for c in range(4):
                    xt[(c, u)] = xpool.tile([128, QB * WP], F16,
                                            tag=f"x{c}", name=f"x{c}")
                    load_batch(xt[(c, u)], c, t, b)

            a_t = [
                apool.tile([128, N_DX, M], F16, tag=f"a{s}", name=f"a{s}")
                for s in range(BS)
            ]
            for s in range(BS):
                nc.sync.dma_start(out=a_t[s][:], in_=a_d[s])

            for u, (t, b) in enumerate(steps):
                if u + 2 < len(steps):
                    tn, bn = steps[u + 2]
                    for c in range(4):
                        xt[(c, u + 2)] = xpool.tile([128, QB * WP], F16,
                                                    tag=f"x{c}", name=f"xn{c}")
                        load_batch(xt[(c, u + 2)], c, tn, bn)
                o_t = opool.tile([128, QB * W], F16, tag="o", name="o")
                for q in range(QB):
                    # the 4 column tiles run 4 strips of image 3c+t in
                    # round-robin; dependent accumulations are 4 apart
                    acc = psum.tile([128, W], F32, tag="ps", name="ps")
                    for j in range(N_DX):
                        for c in range(4):
                            nc.tensor.matmul(
                                acc[32 * c:32 * c + M, :],
                                a_t[c][0:K2, j, :],
                                xt[(c, u)][0:K2,
                                           WP * q + 2 * j:WP * q + 2 * j + W],
                                start=(j == 0),
                                stop=(j == N_DX - 1),
                                tile_position=(0, 32 * c),
                            )
                    nc.vector.tensor_copy(out=o_t[:, q * W:(q + 1) * W],
                                          in_=acc[:])
                # store QB strips per column tile: 128 contiguous rows
                for c in range(4):
                    img = 3 * c + t
                    dv = out_d[img, b * QB * M:(b + 1) * QB * M,
                               :].rearrange("(q p) c -> p q c", q=QB)
                    sv = o_t[32 * c:32 * c + M, :].rearrange(
                        "p (q c) -> p q c", c=W)
                    nc.scalar.dma_start(out=dv, in_=sv)
                for c in range(4):
                    del xt[(c, u)]
    nc.compile()
    return nc


def prepare_in_maps(x: np.ndarray, kern: np.ndarray) -> list:
    # host-side reflection pad, fp16, rows padded to WPH with zero columns
    xpc = np.pad(x, ((0, 0), (0, 0), (P, P), (P, P)), mode="reflect")
    xp = np.zeros((B * C, HP, WPH), dtype=np.float16)
    xp[:, :, :WP] = xpc.reshape(B * C, HP, WP).astype(np.float16)

    # dual-band matrices: lower band = even taps, upper band = odd taps
    kern16 = kern.astype(np.float16)
    a_all = np.zeros((B, 128, N_DX, M), dtype=np.float16)
    m_idx = np.arange(M)
    for dy in range(L):
        a_all[:, m_idx + dy, :, m_idx] = kern16[:, dy, 0::2]
        a_all[:, KG + m_idx + dy, :L // 2, m_idx] = kern16[:, dy, 1::2]

    return [
        {
            "xp": xp[c * NIMG:(c + 1) * NIMG],
            "a": a_all[c * BS:(c + 1) * BS],
        }
        for c in range(N_CORES)
    ]


def kernel(x: np.ndarray, kernel: np.ndarray) -> np.ndarray:
    global _program_cache
    x = np.asarray(x, dtype=np.float32)
    kern = np.asarray(kernel, dtype=np.float32)

    in_maps = prepare_in_maps(x, kern)
    if _program_cache is None:
        _program_cache = _build_program()
    nc = _program_cache

    res = run_bass_kernel_spmd(nc, in_maps, core_ids=list(range(N_CORES)))
    out = np.concatenate([r["out"] for r in res.results], axis=0)
    return out.reshape(B, C, H, W).astype(np.float32)


# revision 7
# speedup vs baseline: 1.9467x; 1.9467x over previous
"""BatchBlur: depthwise 15x15 conv with per-sample kernels, reflection pad 7.

x: (32, 3, 512, 512) f32, kernel: (32, 15, 15) f32 -> out (32, 3, 512, 512) f32.

Strategy: pure data parallel over batch, 4 samples (12 channel-images) per
core on 8 cores. Host: reflection-pad x to (., 526, 526), store rows padded
to 528 with zero columns, cast to fp16, and build dual-band matrices
A[s, k, j, m]: for k < 64, A = kern[s, k-m, 2j]; for k >= 64,
A = kern[s, k-64-m, 2j+1] (band condition 0 <= dy < 15; j = 7 upper band
is zero).

Device: measured PE law on TRN2 - a matmul costs
  N * max(K_pad/256, M_pad/128) cycles   (K_pad, M_pad padded to 32/64/128)
i.e. the rhs feed sustains 256 fp16 elements/cycle and the PSUM port 128
writes/cycle - and back-to-back matmuls accumulating into the same PSUM
region stall ~112 extra cycles unless >= 4 independent chains are
interleaved. The optimal banded mapping is therefore M=50 output rows,
dual-band K=128 (rows at column offsets 0 and +1 -> one stream covers taps
2j and 2j+1, 8 streams for 15 taps), both constraints saturated at 2
cols/cycle:
  out[m, n] += sum_k A[k, j, m] * rhs[k, n + 2j]
Two images run in the two 64-column halves of the PE (tile_position
(0,0)/(0,64)), and TWO strips of each image are processed concurrently in
separate PSUM banks, giving 4 interleaved accumulation chains - dependent
matmuls 4 apart, drain fully hidden. 1056 matmuls x 256 cycles ~= 113 us.
Strips: rows 0..499 in five 2x50-row doublets plus one final strip at
r0=462 (rows 462..525 = the padded end) whose store is sliced to rows
500..511. Eviction casts f32 PSUM to fp16 (DVE) and stores fp16; the host
casts back to f32 (adds ~4e-4 relative error).
"""
import os
import sys

for _p in ("/opt/trn_rl_repo", "/root/.axon_site/_ro/trn_rl_repo"):
    if _p not in sys.path and os.path.isdir(_p):
        sys.path.insert(0, _p)

import numpy as np

import concourse.bass as bass
import concourse.mybir as mybir
import concourse.tile as tile
from concourse import bacc
from concourse.bass_utils import run_bass_kernel_spmd

L = 15           # blur kernel size
P = L // 2       # reflection pad
B, C, H, W = 32, 3, 512, 512
N_CORES = 8
BS = B // N_CORES            # samples per core (4)
NIMG = BS * C                # channel images per core (12)
HP, WP = H + 2 * P, W + 2 * P  # 526
WPH = WP + 2                 # host row pitch: +2 zero cols so the +1-shifted
                             # band reads defined data at its last column
M_STRIP = 50                 # output rows per strip (dual-band: 2*(50+14)=128)
K_GRP = M_STRIP + L - 1      # 64 input rows per band group
N_DX = (L + 1) // 2          # 8 streams (two taps each; last is single)
R0_LAST = HP - K_GRP         # 462: final strip ends exactly at padded edge
N_WARMUP = 100               # dummy matmuls to release the HAM clock gate

F16 = mybir.dt.float16
F32 = mybir.dt.float32

_program_cache = None


def _build_program():
    nc = bacc.Bacc("TRN2", target_bir_lowering=False, debug=False)
    xp_d = nc.dram_tensor("xp", [NIMG, HP, WPH], F16,
                          kind="ExternalInput").ap()
    a_d = nc.dram_tensor("a", [BS, 128, N_DX, M_STRIP], F16,
                         kind="ExternalInput").ap()
    out_d = nc.dram_tensor("out", [NIMG, H, W], F16,
                           kind="ExternalOutput").ap()

    def load_strip2(t, img, r0):
        # one DMA per band brings rows for strips r0 and r0+50 (free-dim
        # blocks 0/1). Lower band (Sync queue) at column offset 0, upper
        # band (GpSimd queue) at offset +1 - the +1 band's last column
        # reads the host's zero padding, so everything is defined.
        base = (img * HP + r0) * WPH
        nc.sync.dma_start(
            out=t[0:K_GRP, :].rearrange("p (q c) -> p q c", c=WP),
            in_=bass.AP(xp_d.tensor, base,
                        [[WPH, K_GRP], [M_STRIP * WPH, 2], [1, WP]]))
        nc.gpsimd.dma_start(
            out=t[K_GRP:2 * K_GRP, :].rearrange("p (q c) -> p q c", c=WP),
            in_=bass.AP(xp_d.tensor, base + 1,
                        [[WPH, K_GRP], [M_STRIP * WPH, 2], [1, WP]]))

    def load_strip1(t, img, r0):
        base = (img * HP + r0) * WPH
        nc.sync.dma_start(
            out=t[0:K_GRP, 0:WP],
            in_=bass.AP(xp_d.tensor, base, [[WPH, K_GRP], [1, WP]]))
        nc.gpsimd.dma_start(
            out=t[K_GRP:2 * K_GRP, 0:WP],
            in_=bass.AP(xp_d.tensor, base + 1, [[WPH, K_GRP], [1, WP]]))

    with tile.TileContext(nc) as tc:
        with (
            tc.tile_pool(name="aconst", bufs=1) as apool,
            tc.tile_pool(name="warm", bufs=1) as wpool,
            tc.tile_pool(name="xin", bufs=6) as xpool,
            tc.tile_pool(name="oout", bufs=4) as opool,
            tc.tile_pool(name="psum", bufs=6, space="PSUM") as psum,
            tc.tile_pool(name="psumw", bufs=1, space="PSUM") as psumw,
        ):
            # HAM warm-up: a burst of matmuls on a zeroed scratch tile
            # releases the PE clock gate while the first input DMAs are in
            # flight.
            wsrc = wpool.tile([128, 64], mybir.dt.bfloat16)
            nc.gpsimd.memset(wsrc[:], 0.0)
            wacc = psumw.tile([64, 64], F32)
            for _ in range(N_WARMUP):
                nc.tensor.matmul(wacc[:], wsrc[:, :64], wsrc[:], start=True,
                                 stop=True)

            # first doublet's image rows: issued before the A load so the
            # DMA queues deliver the first matmuls' dependencies earliest
            xp_first = []
            for img in range(2):
                t = xpool.tile([128, 2 * WP], F16, tag="xp2", name=f"xpf{img}")
                load_strip2(t, img, 0)
                xp_first.append(t)

            a_t = [
                apool.tile([128, N_DX, M_STRIP], F16, tag=f"a{s}",
                           name=f"a{s}")
                for s in range(BS)
            ]
            nc.sync.dma_start(out=a_t[0][:], in_=a_d[0])

            a_loaded = 0
            for pair in range(NIMG // 2):
                img_a, img_b = 2 * pair, 2 * pair + 1
                smp_a, smp_b = img_a // C, img_b // C
                for s_need in ((2 * pair + 2) // C, (2 * pair + 3) // C):
                    if s_need < BS and s_need > a_loaded:
                        nc.sync.dma_start(out=a_t[s_need][:], in_=a_d[s_need])
                        a_loaded = s_need

                # five strip-doublets (rows 0..499) + one single overlap
                # strip at r0=462 storing rows 500..511
                for du in range(6):
                    if du < 5:
                        r0 = 100 * du
                        if pair == 0 and du == 0:
                            xa, xb = xp_first
                        else:
                            xa = xpool.tile([128, 2 * WP], F16, tag="xp2",
                                            name="xa")
                            load_strip2(xa, img_a, r0)
                            xb = xpool.tile([128, 2 * WP], F16, tag="xp2",
                                            name="xb")
                            load_strip2(xb, img_b, r0)
                        o_t = opool.tile([128, 2 * W], F16)
                        # two strips (free-dim blocks of xa/xb) accumulate in
                        # two PSUM banks; with the two PE column halves that
                        # is 4 interleaved chains - dependent matmuls are 4
                        # apart and the accumulation drain is hidden
                        accs = [psum.tile([128, W], F32, tag="ps", name="ps")
                                for _ in range(2)]
                        for j in range(N_DX):
                            for sub in range(2):
                                base = sub * WP
                                nc.tensor.matmul(
                                    accs[sub][0:M_STRIP],
                                    a_t[smp_a][:, j, :],
                                    xa[:, base + 2 * j:base + 2 * j + W],
                                    start=(j == 0),
                                    stop=(j == N_DX - 1),
                                    tile_position=(0, 0),
                                )
                                nc.tensor.matmul(
                                    accs[sub][64:64 + M_STRIP],
                                    a_t[smp_b][:, j, :],
                                    xb[:, base + 2 * j:base + 2 * j + W],
                                    start=(j == 0),
                                    stop=(j == N_DX - 1),
                                    tile_position=(0, 64),
                                )
                        for sub in range(2):
                            nc.vector.tensor_copy(
                                out=o_t[:, sub * W:(sub + 1) * W],
                                in_=accs[sub][:])
                        # one store per image covers both strips (100
                        # contiguous output rows; non-overlapping views)
                        dva = out_d[img_a, r0:r0 + 2 * M_STRIP, :].rearrange(
                            "(q p) c -> p q c", q=2)
                        dvb = out_d[img_b, r0:r0 + 2 * M_STRIP, :].rearrange(
                            "(q p) c -> p q c", q=2)
                        sva = o_t[0:M_STRIP, :].rearrange(
                            "p (q c) -> p q c", c=W)
                        svb = o_t[64:64 + M_STRIP, :].rearrange(
                            "p (q c) -> p q c", c=W)
                        nc.scalar.dma_start(out=dva, in_=sva)
                        nc.scalar.dma_start(out=dvb, in_=svb)
                    else:
                        r0 = R0_LAST  # 462
                        lo = 10 * M_STRIP - r0  # store rows 500..511 only
                        xa = xpool.tile([128, WP], F16, tag="xp_t", name="xa1")
                        load_strip1(xa, img_a, r0)
                        xb = xpool.tile([128, WP], F16, tag="xp_t", name="xb1")
                        load_strip1(xb, img_b, r0)
                        acc = psum.tile([128, W], F32, tag="ps1", name="ps1",
                                        bufs=1)
                        for j in range(N_DX):
                            nc.tensor.matmul(
                                acc[0:M_STRIP], a_t[smp_a][:, j, :],
                                xa[:, 2 * j:2 * j + W], start=(j == 0),
                                stop=(j == N_DX - 1), tile_position=(0, 0))
                            nc.tensor.matmul(
                                acc[64:64 + M_STRIP], a_t[smp_b][:, j, :],
                                xb[:, 2 * j:2 * j + W], start=(j == 0),
                                stop=(j == N_DX - 1), tile_position=(0, 64))
                        o_s = opool.tile([128, W], F16, tag="o1", name="o1")
                        nc.vector.tensor_copy(out=o_s[:], in_=acc[:])
                        nc.scalar.dma_start(
                            out=out_d[img_a, r0 + lo:r0 + M_STRIP, :],
                            in_=o_s[lo:M_STRIP])
                        nc.scalar.dma_start(
                            out=out_d[img_b, r0 + lo:r0 + M_STRIP, :],
                            in_=o_s[64 + lo:64 + M_STRIP])
    nc.compile()
    return nc


def prepare_in_maps(x: np.ndarray, kern: np.ndarray) -> list:
    # host-side reflection pad, fp16, rows padded to WPH with zero columns
    xpc = np.pad(x, ((0, 0), (0, 0), (P, P), (P, P)), mode="reflect")
    xp = np.zeros((B * C, HP, WPH), dtype=np.float16)
    xp[:, :, :WP] = xpc.reshape(B * C, HP, WP).astype(np.float16)

    # dual-band matrices: lower band = even taps, upper band = odd taps
    kern16 = kern.astype(np.float16)
    a_all = np.zeros((B, 128, N_DX, M_STRIP), dtype=np.float16)
    m_idx = np.arange(M_STRIP)
    for dy in range(L):
        a_all[:, m_idx + dy, :, m_idx] = kern16[:, dy, 0::2]
        a_all[:, K_GRP + m_idx + dy, :L // 2, m_idx] = kern16[:, dy, 1::2]

    return [
        {
            "xp": xp[c * NIMG:(c + 1) * NIMG],
            "a": a_all[c * BS:(c + 1) * BS],
        }
        for c in range(N_CORES)
    ]


def kernel(x: np.ndarray, kernel: np.ndarray) -> np.ndarray:
    global _program_cache
    x = np.asarray(x, dtype=np.float32)
    kern = np.asarray(kernel, dtype=np.float32)

    in_maps = prepare_in_maps(x, kern)
    if _program_cache is None:
        _program_cache = _build_program()
    nc = _program_cache

    res = run_bass_kernel_spmd(nc, in_maps, core_ids=list(range(N_CORES)))
    out = np.concatenate([r["out"] for r in res.results], axis=0)
    return out.reshape(B, C, H, W).astype(np.float32)
